# revision 4
# baseline (speedup 1.0000x reference)
"""AdaptiveSparsityAttention TRN2 kernel (8 NeuronCores, SPMD data-parallel).

Problem (B=2, S=1024, D=512, H=2 heads, dh=256, hidden=128):
  q,k,v = x@Wq, x@Wk, x@Wv (split 2 heads); scores = q@k^T/16
  a_i = q_mean@W1[:dh]+b1, c_j = k_mean@W1[dh:]
  z[i,j] = W2 . relu(a_i + c_j)          (sigmoid(z+b2)>0.5  <=>  z > -b2)
  attn = softmax(mask(scores));  out = (attn@v)@Wo + bo

Sharding: 8 cores = 2 batches x 4 query-chunks of 256 rows. Each core
computes its output chunk fully locally, no collectives.

Precision structure (measured, numpy sims):
  - z MUST be fp32-exact: z std 0.022, ~46% fill, threshold 0. bf16-level
    z error flips ~0.1% of mask bits -> 5.8e-2 L2 (FAILS 2e-2 gate);
    fp16 -> 1.5e-2 (too marginal). fp32 mask matches jax ref at 1.4e-3.
  - Everything downstream of the mask is bf16-safe: bf16 q/k/v/scores/
    attn-weights/out-proj measured 4.1e-3 total vs jax ref (5x margin).
    bf16 (vs baseline's float32r) buys FWL fast weight loads on PE,
    2-4x DVE modes, and half the DMA bytes.

Performance structure (per core, from perfetto traces):
  - z stream: 256 steps (1 query row each) of [T-tile produce on DVE
    (~805ns) or ACT (~1054ns)] + 2 fp32 matmul calls (wsel delta trick,
    4-way col-tiled, 4 LDW + 4 MM instrs/step, LDW ~110ns each).
    Pace ~470-490ns/step, co-bound by dual-engine T production (~456)
    and the LDW chain (~440). This is near the structural floor: fp32
    MMs cannot share LDWs in bass, and exact T production is elementwise
    fp32 work on DVE/ACT (tensor_scalar runs 2 elem/cyc/lane via 2x_2P).
  - All other work (QKV projections, scores, softmax, AV, out-proj) is
    emitted as "fillers" interleaved into the two z-block windows so PE/
    DVE/ACT gaps between phases vanish.
  - Input DMAs are spread across 5 engine queues (sync/tensor/gpsimd/
    scalar/vector) instead of serializing ~23us on sync.
"""

import sys

if "/opt/trn_rl_repo" not in sys.path:
    sys.path.insert(0, "/opt/trn_rl_repo")

import numpy as np
import ml_dtypes

import concourse.bass as bass  # noqa: F401
import concourse.tile as tile
from concourse import bacc, mybir
from concourse.bass_utils import run_bass_kernel_spmd

F32 = mybir.dt.float32
BF16 = mybir.dt.bfloat16
AL = mybir.AluOpType
AF = mybir.ActivationFunctionType

B, S, D = 2, 1024, 512
DH = D // 2          # 256 per-head dim
HID = 128            # predictor hidden
NCHUNK = S // 4      # 256 query rows per core
P = 128

# knobs (test.py may override before first kernel() call)
CONFIG = {
    "trace": False,
    "tmpdir": None,
    # T-producer rotation per 16 rows: 'v'=DVE, 's'=ACT
    "tpat": ["v", "s", "v", "s", "v", "v", "s", "v", "s", "v", "v", "s", "v", "s", "v", "s"],
    "t_bufs": 11,
    "fill_start": 8,   # first z step that may carry a filler
    "fill_end": 124,   # last z step that may carry a filler
}

_STATE = {}


def _emit(tc, nc, t):
    sl512 = [slice(0, 512), slice(512, 1024)]

    with tc.tile_pool(name="big", bufs=1) as big:
        # ---- persistent residents ----
        mq_s = big.tile([P, 4, HID], F32, name="mq_s")
        mk_s = big.tile([P, 4, HID], F32, name="mk_s")
        b1_s = big.tile([P, 1], F32, name="b1_s")
        thr_s = big.tile([P, 1], F32, name="thr_s")
        wsel_s = big.tile([P, 64], F32, name="wsel_s")
        bo_s = big.tile([1, D], BF16, name="bo_s")
        one_s = big.tile([1, P], BF16, name="one_s")
        ident = big.tile([P, P], BF16, name="ident")

        at_s = big.tile([P, NCHUNK], F32, name="at_s")    # a^T + b1, [h, i]
        nat_s = big.tile([P, NCHUNK], F32, name="nat_s")  # -(a^T + b1)
        ct_s = big.tile([P, S], F32, name="ct_s")          # c^T, [h, j]

        xbq_s = big.tile([P, 4, NCHUNK], BF16, name="xbq_s")  # x^T qchunk bf16
        xbt_s = big.tile([P, 4, S], BF16, name="xbt_s")       # x^T bf16
        wqb_s = big.tile([P, 4, D], BF16, name="wqb_s")       # Wq/16 bf16
        wkb_s = big.tile([P, 4, D], BF16, name="wkb_s")
        wvb_s = big.tile([P, 4, D], BF16, name="wvb_s")
        wob_s = big.tile([P, 4, D], BF16, name="wob_s")
        qt_s = big.tile([P, 4, NCHUNK], BF16, name="qt_s")  # q^T/16 [dout, i]
        kt_s = big.tile([P, 4, S], BF16, name="kt_s")       # k^T [dout, j]
        v_s = big.tile([P, 8, D], BF16, name="v_s")         # v [j(8 tiles), d]
        otr_s = big.tile([P, 4, NCHUNK], BF16, name="otr_s")  # out^T [d, i]
        mask_s = [big.tile([P, S], BF16, name=f"mask{b_}") for b_ in range(2)]

        # ---- input DMAs, spread across 5 engine queues ----
        # critical path first on each queue: x fp32 (a/c), then x bf16,
        # then weights (needed later as fillers consume them).
        nc.sync.dma_start(b1_s[:], t["b1c"])
        nc.sync.dma_start(thr_s[:], t["thr"])
        nc.sync.dma_start(wsel_s[:], t["wsel32"])
        for t_ in range(4):
            nc.sync.dma_start(mq_s[:, t_, :], t["mq"][128 * t_ : 128 * (t_ + 1), :])
            nc.sync.dma_start(mk_s[:, t_, :], t["mk"][128 * t_ : 128 * (t_ + 1), :])
        nc.gpsimd.dma_start(ident[:], t["identb"])
        nc.gpsimd.dma_start(bo_s[:], t["bo2"])
        nc.gpsimd.dma_start(one_s[:], t["one1"])

        with (
            tc.tile_pool(name="pjp", bufs=1, space="PSUM") as pjp,
            tc.tile_pool(name="zps", bufs=1, space="PSUM") as zpsp,
        ):
            # ---------------- stage A (exact a/c; transient f32 staging) ----
            with tc.tile_pool(name="stageA", bufs=1) as sa:
                xq_s = sa.tile([P, 4, NCHUNK], F32, name="xq_s")
                xt_s = sa.tile([P, 4, S], F32, name="xt_s")
                # fp32 x chunks: spread over the 3 DMA-capable queues
                qs = [nc.gpsimd, nc.scalar, nc.sync, nc.gpsimd]
                for t_ in range(4):
                    qs[t_].dma_start(xq_s[:, t_, :], t["xqT"][128 * t_ : 128 * (t_ + 1), :])
                for t_ in range(4):
                    qs[t_].dma_start(xt_s[:, t_, :], t["xT"][128 * t_ : 128 * (t_ + 1), :])
                # bf16 x + weights, spread; order matters within a queue
                for t_ in range(4):
                    qs[t_].dma_start(xbq_s[:, t_, :], t["xbqT"][128 * t_ : 128 * (t_ + 1), :])
                for t_ in range(4):
                    qs[t_].dma_start(xbt_s[:, t_, :], t["xbT"][128 * t_ : 128 * (t_ + 1), :])
                for wi, (nm, dst) in enumerate(
                    [("wq", wqb_s), ("wk", wkb_s), ("wv", wvb_s), ("wo", wob_s)]
                ):
                    for t_ in range(4):
                        qs[(wi + t_) % 4].dma_start(
                            dst[:, t_, :], t[nm][128 * t_ : 128 * (t_ + 1), :]
                        )

                # exact a (fp32 matmuls)
                at_ps = pjp.tile([P, NCHUNK], F32, tag="vps", bufs=2, name="at_ps")
                for dt_ in range(4):
                    nc.tensor.matmul(
                        at_ps[:], mq_s[:, dt_, :], xq_s[:, dt_, :],
                        start=(dt_ == 0), stop=(dt_ == 3),
                    )
                nc.vector.tensor_scalar(at_s[:], at_ps[:], b1_s[:], None, AL.add)
                nc.vector.tensor_scalar(nat_s[:], at_s[:], -1.0, None, AL.mult)

                # exact c (fp32 matmuls)
                for jc in range(2):
                    ct_ps = pjp.tile([P, 512], F32, tag="vps", bufs=2, name="ct_ps")
                    for dt_ in range(4):
                        nc.tensor.matmul(
                            ct_ps[:], mk_s[:, dt_, :], xt_s[:, dt_, sl512[jc]],
                            start=(dt_ == 0), stop=(dt_ == 3),
                        )
                    nc.scalar.copy(ct_s[:, sl512[jc]], ct_ps[:])

            # ---------------- z blocks with interleaved fillers ------------
            with (
                tc.tile_pool(name="Tp", bufs=CONFIG["t_bufs"]) as Tp,
                tc.tile_pool(name="work", bufs=2) as work,
            ):
                # ---- filler pieces (bf16 PE work + copies) ----
                def f_qt(dout):
                    def go():
                        qt_ps = pjp.tile([P, NCHUNK], F32, tag="vps", bufs=2, name="qt_ps")
                        for dt_ in range(4):
                            nc.tensor.matmul(
                                qt_ps[:], wqb_s[:, dt_, 128 * dout : 128 * (dout + 1)],
                                xbq_s[:, dt_, :], start=(dt_ == 0), stop=(dt_ == 3),
                            )
                        if dout % 2 == 0:
                            nc.vector.tensor_copy(qt_s[:, dout, :], qt_ps[:])
                        else:
                            nc.scalar.copy(qt_s[:, dout, :], qt_ps[:])
                    return go

                def f_kt(dout, jc):
                    def go():
                        kt_ps = pjp.tile([P, 512], F32, tag="vps", bufs=2, name="kt_ps")
                        for dt_ in range(4):
                            nc.tensor.matmul(
                                kt_ps[:],
                                wkb_s[:, dt_, 128 * dout : 128 * (dout + 1)],
                                xbt_s[:, dt_, sl512[jc]],
                                start=(dt_ == 0), stop=(dt_ == 3),
                            )
                        if (dout + jc) % 2 == 0:
                            nc.vector.tensor_copy(kt_s[:, dout, sl512[jc]], kt_ps[:])
                        else:
                            nc.scalar.copy(kt_s[:, dout, sl512[jc]], kt_ps[:])
                    return go

                def f_v(jt):
                    def go():
                        v_ps = pjp.tile([P, D], F32, tag="vps", bufs=2, name="v_ps")
                        for dt_ in range(4):
                            nc.tensor.matmul(
                                v_ps[:], xbt_s[:, dt_, 128 * jt : 128 * (jt + 1)],
                                wvb_s[:, dt_, :], start=(dt_ == 0), stop=(dt_ == 3),
                            )
                        if jt % 2 == 0:
                            nc.vector.tensor_copy(v_s[:, jt, :], v_ps[:])
                        else:
                            nc.scalar.copy(v_s[:, jt, :], v_ps[:])
                    return go

                # per-head attnT accumulators (written in ti halves)
                att_sb = [
                    [
                        work.tile([P, NCHUNK], BF16, tag="attnT", bufs=16,
                                  name=f"a_sb{h}_{jt}")
                        for jt in range(8)
                    ]
                    for h in range(2)
                ]

                def attn_half_pieces(h, ti):
                    """softmax(masked scores) for rows [128*ti,128*(ti+1)) of
                    head h, transposed into att_sb[h][*][:, 128*ti:]. Returns
                    a list of closures (emission units)."""
                    st = {}

                    def sc(jc):
                        def go():
                            scp = pjp.tile([P, 512], F32, tag="vps", bufs=2,
                                           name=f"sc{h}{ti}{jc}")
                            st[f"sc{jc}"] = scp
                            for dt_ in range(2):
                                nc.tensor.matmul(
                                    scp[:],
                                    qt_s[:, 2 * h + dt_, 128 * ti : 128 * (ti + 1)],
                                    kt_s[:, 2 * h + dt_, sl512[jc]],
                                    start=(dt_ == 0), stop=(dt_ == 1),
                                )
                        return go

                    def ex(jc):
                        def go():
                            if jc == 0:
                                st["e"] = work.tile([P, S], BF16, tag="e", name="e")
                            # scores/16 bounded (|sc|<~7): exp cannot overflow,
                            # rowmax subtraction dropped (identical result)
                            nc.scalar.activation(
                                st["e"][:, sl512[jc]], st[f"sc{jc}"][:], AF.Exp
                            )
                        return go

                    def sm():
                        e = st["e"]
                        em = work.tile([P, S], BF16, tag="em", name="em")
                        st["em"] = em
                        ssum = work.tile([P, 1], F32, tag="ssum", name="ssum")
                        nc.vector.scalar_tensor_tensor(
                            em[:], e[:], 0.0, mask_s[ti][:], AL.add, AL.mult,
                            accum_out=ssum[:],
                        )
                        # fully-masked rows: reference = uniform 1/1024.
                        # ind=[s==0]; attn = (em + ind) / (s + 1024*ind)
                        ind = work.tile([P, 1], F32, tag="ind", name="ind")
                        nc.vector.tensor_scalar(ind[:], ssum[:], 0.0, None, AL.is_equal)
                        s2 = work.tile([P, 1], F32, tag="s2", name="s2")
                        nc.vector.tensor_scalar(s2[:], ind[:], 1024.0, ssum[:], AL.mult, AL.add)
                        rinv = work.tile([P, 1], F32, tag="rinv", name="rinv")
                        nc.vector.reciprocal(rinv[:], s2[:])
                        nc.vector.tensor_scalar(em[:], em[:], ind[:], rinv[:], AL.add, AL.mult)

                    def tp(half):
                        def go():
                            for jt in range(4 * half, 4 * half + 4):
                                tp_ps = pjp.tile([P, P], BF16, tag="tp", bufs=2, name="tp_ps")
                                nc.tensor.transpose(
                                    tp_ps[:], st["em"][:, 128 * jt : 128 * (jt + 1)], ident[:]
                                )
                                dst = att_sb[h][jt][:, 128 * ti : 128 * (ti + 1)]
                                if jt % 2 == 0:
                                    nc.vector.tensor_copy(dst, tp_ps[:])
                                else:
                                    nc.scalar.copy(dst, tp_ps[:])
                        return go

                    return [sc(0), ex(0), sc(1), ex(1), sm, tp(0), tp(1)]

                def emit_av(h):
                    for dt_ in range(2):
                        ot_ps = pjp.tile([P, NCHUNK], F32, tag="vps", bufs=2, name="ot_ps")
                        for jt in range(8):
                            nc.tensor.matmul(
                                ot_ps[:],
                                v_s[:, jt, 256 * h + 128 * dt_ : 256 * h + 128 * (dt_ + 1)],
                                att_sb[h][jt][:],
                                start=(jt == 0), stop=(jt == 7),
                            )
                        if dt_ == 0:
                            nc.vector.tensor_copy(otr_s[:, 2 * h + dt_, :], ot_ps[:])
                        else:
                            nc.scalar.copy(otr_s[:, 2 * h + dt_, :], ot_ps[:])

                def emit_zblock(blk, fillers=()):
                    # z accumulation for 128 query rows; col groups MUST cycle
                    # (g=step%4): serial same-group fp32 mms run at 2cyc/col,
                    # cycling pipelines passes across the 4 array quadrants.
                    zp = zpsp.tile([P, S], F32, tag="z", bufs=2, name=f"zp{blk}")
                    nf = len(fillers)
                    fi = 0
                    a_, b_ = CONFIG["fill_start"], CONFIG["fill_end"]
                    for step in range(128):
                        k, g = step // 4, step % 4
                        i = 32 * g + k
                        ii = blk * 128 + i
                        T = Tp.tile([P, S], F32, tag="T", name=f"T{ii}")
                        eng = CONFIG["tpat"][ii % 16]
                        if eng == "v":
                            # relu(ct + a) as max(ct, -a) + a  ((add,max) hits a
                            # slow DVE path; (max,add) measured ~805ns)
                            nc.vector.tensor_scalar(
                                T[:], ct_s[:], nat_s[:, ii : ii + 1],
                                at_s[:, ii : ii + 1], AL.max, AL.add,
                            )
                        else:
                            nc.scalar.activation(
                                T[:], ct_s[:], AF.Relu, bias=at_s[:, ii : ii + 1]
                            )
                        for jc in range(2):
                            nc.tensor.matmul(
                                zp[32 * g : 32 * g + 32, sl512[jc]],
                                wsel_s[:, 32 - k : 64 - k],
                                T[:, sl512[jc]],
                                start=(k == 0), stop=(k == 31),
                                tile_position=(0, 32 * g),
                                skip_group_check=True,
                            )
                        while fi < nf and a_ + ((b_ - a_) * fi) // max(nf - 1, 1) <= step:
                            fillers[fi]()
                            fi += 1
                    while fi < nf:
                        fillers[fi]()
                        fi += 1
                    nc.vector.tensor_scalar(
                        mask_s[blk][:], zp[:], thr_s[:], None, AL.is_gt,
                    )

                # ---- emission schedule ----
                blk0_fill = (
                    [f_qt(d) for d in range(4)]
                    + [f_kt(d, jc) for d in range(4) for jc in range(2)]
                    + [f_v(jt) for jt in range(8)]
                )
                emit_zblock(0, blk0_fill)

                blk1_fill = attn_half_pieces(0, 0) + attn_half_pieces(1, 0)
                emit_zblock(1, blk1_fill)

                # ---- tail: attn ti=1 for both heads, AV, out-proj ----
                p0 = attn_half_pieces(0, 1)
                p1 = attn_half_pieces(1, 1)
                p0[0](); p0[1](); p0[2](); p0[3]()   # sc0/exp0/sc1/exp1 head0
                p1[0](); p1[1]()                     # head1 sc0/exp0
                p0[4]()                              # head0 softmax (DVE)
                p1[2](); p1[3]()                     # head1 sc1/exp1
                p0[5](); p0[6]()                     # head0 transposes
                p1[4]()                              # head1 softmax
                emit_av(0)
                p1[5](); p1[6]()                     # head1 transposes
                emit_av(1)

                # stage F: output projection (bf16; bo/ones exact in bf16)
                for ti in range(2):
                    o_ps = pjp.tile([P, D], F32, tag="vps", bufs=2, name="o_ps")
                    nc.tensor.matmul(o_ps[:], one_s[:], bo_s[:], start=True, stop=False)
                    for dt_ in range(4):
                        nc.tensor.matmul(
                            o_ps[:], otr_s[:, dt_, 128 * ti : 128 * (ti + 1)],
                            wob_s[:, dt_, :], start=False, stop=(dt_ == 3),
                        )
                    o_sb = work.tile([P, D], F32, tag="osb", bufs=2, name="o_sb")
                    nc.vector.tensor_copy(o_sb[:], o_ps[:])
                    nc.sync.dma_start(t["out"][128 * ti : 128 * (ti + 1), :], o_sb[:])


def _build():
    if "nc" in _STATE:
        return _STATE["nc"]
    nc = bacc.Bacc(
        "TRN2", target_bir_lowering=False, debug=False, enable_asserts=True,
        num_devices=8,
    )
    t = {}
    t["xT"] = nc.dram_tensor("xT", [D, S], F32, kind="ExternalInput").ap()
    t["xqT"] = nc.dram_tensor("xqT", [D, NCHUNK], F32, kind="ExternalInput").ap()
    t["xbT"] = nc.dram_tensor("xbT", [D, S], BF16, kind="ExternalInput").ap()
    t["xbqT"] = nc.dram_tensor("xbqT", [D, NCHUNK], BF16, kind="ExternalInput").ap()
    t["wq"] = nc.dram_tensor("wq", [D, D], BF16, kind="ExternalInput").ap()
    t["wk"] = nc.dram_tensor("wk", [D, D], BF16, kind="ExternalInput").ap()
    t["wv"] = nc.dram_tensor("wv", [D, D], BF16, kind="ExternalInput").ap()
    t["wo"] = nc.dram_tensor("wo", [D, D], BF16, kind="ExternalInput").ap()
    t["mq"] = nc.dram_tensor("mq", [D, HID], F32, kind="ExternalInput").ap()
    t["mk"] = nc.dram_tensor("mk", [D, HID], F32, kind="ExternalInput").ap()
    t["b1c"] = nc.dram_tensor("b1c", [P, 1], F32, kind="ExternalInput").ap()
    t["thr"] = nc.dram_tensor("thr", [P, 1], F32, kind="ExternalInput").ap()
    t["wsel32"] = nc.dram_tensor("wsel32", [P, 64], F32, kind="ExternalInput").ap()
    t["bo2"] = nc.dram_tensor("bo2", [1, D], BF16, kind="ExternalInput").ap()
    t["one1"] = nc.dram_tensor("one1", [1, P], BF16, kind="ExternalInput").ap()
    t["identb"] = nc.dram_tensor("identb", [P, P], BF16, kind="ExternalInput").ap()
    t["out"] = nc.dram_tensor("out", [NCHUNK, D], F32, kind="ExternalOutput").ap()

    with tile.TileContext(nc) as tc:
        _emit(tc, nc, t)
    nc.compile()
    _STATE["nc"] = nc
    return nc


def _prep_in_maps(inputs):
    bf16 = ml_dtypes.bfloat16
    x = np.ascontiguousarray(np.asarray(inputs["x"], np.float32))
    Wq = np.asarray(inputs["Wq"], np.float32)
    Wk = np.asarray(inputs["Wk"], np.float32)
    Wv = np.asarray(inputs["Wv"], np.float32)
    Wo = np.asarray(inputs["Wo"], np.float32)
    bo = np.asarray(inputs["bo"], np.float32)
    W1 = np.asarray(inputs["W1"], np.float64)
    b1 = np.asarray(inputs["b1"], np.float32)
    W2 = np.asarray(inputs["W2"], np.float32)
    b2 = np.asarray(inputs["b2"], np.float32)

    wq_m = 0.5 * (Wq[:, :DH].astype(np.float64) + Wq[:, DH:].astype(np.float64))
    wk_m = 0.5 * (Wk[:, :DH].astype(np.float64) + Wk[:, DH:].astype(np.float64))
    Mq = np.ascontiguousarray((wq_m @ W1[:DH]).astype(np.float32))
    Mk = np.ascontiguousarray((wk_m @ W1[DH:]).astype(np.float32))

    wsel32 = np.zeros((P, 64), np.float32)
    wsel32[:, 32] = W2[:, 0]
    b1c = np.ascontiguousarray(b1.reshape(P, 1))
    thr = np.full((P, 1), -float(b2[0]), np.float32)

    shared = dict(
        wq=np.ascontiguousarray((Wq / 16.0).astype(bf16)),
        wk=np.ascontiguousarray(Wk.astype(bf16)),
        wv=np.ascontiguousarray(Wv.astype(bf16)),
        wo=np.ascontiguousarray(Wo.astype(bf16)),
        mq=Mq, mk=Mk, b1c=b1c, thr=thr, wsel32=wsel32,
        bo2=np.ascontiguousarray(bo.reshape(1, D).astype(bf16)),
        one1=np.ones((1, P), bf16),
        identb=np.eye(P, dtype=bf16),
    )
    in_maps = []
    xT = [np.ascontiguousarray(x[b].T) for b in range(B)]
    xbT = [np.ascontiguousarray(xT[b].astype(bf16)) for b in range(B)]
    for c in range(8):
        b, i0 = c // 4, (c % 4) * NCHUNK
        m = dict(shared)
        m["xT"] = xT[b]
        m["xbT"] = xbT[b]
        xq = np.ascontiguousarray(x[b, i0 : i0 + NCHUNK].T)
        m["xqT"] = xq
        m["xbqT"] = np.ascontiguousarray(xq.astype(bf16))
        in_maps.append(m)
    return in_maps


def kernel(**inputs):
    nc = _build()
    in_maps = _prep_in_maps(inputs)
    res = run_bass_kernel_spmd(
        nc, in_maps, core_ids=list(range(8)),
        trace=CONFIG["trace"], tmpdir=CONFIG["tmpdir"],
    )
    _STATE["last_result"] = res
    out = np.empty((B, S, D), np.float32)
    for c in range(8):
        b, i0 = c // 4, (c % 4) * NCHUNK
        out[b, i0 : i0 + NCHUNK] = res.results[c]["out"]
    return out


# revision 10
# speedup vs baseline: 1.0624x; 1.0624x over previous
"""AdaptiveSparsityAttention TRN2 kernel (8 NeuronCores, SPMD data-parallel).

Problem (B=2, S=1024, D=512, H=2 heads, dh=256, hidden=128):
  q,k,v = x@Wq, x@Wk, x@Wv (split 2 heads); scores = q@k^T/16
  a_i = q_mean@W1[:dh]+b1, c_j = k_mean@W1[dh:]
  z[i,j] = W2 . relu(a_i + c_j)          (sigmoid(z+b2)>0.5  <=>  z > -b2)
  attn = softmax(mask(scores));  out = (attn@v)@Wo + bo

Sharding: 8 cores = 2 batches x 4 query-chunks of 256 rows. Each core
computes its output chunk fully locally, no collectives.

Precision structure (measured, numpy sims):
  - z MUST be fp32-exact: z std 0.022, ~46% fill, threshold 0. bf16-level
    z error flips ~0.1% of mask bits -> 5.8e-2 L2 (FAILS 2e-2 gate);
    fp16 -> 1.5e-2 (too marginal). fp32 mask matches jax ref at 1.4e-3.
  - Everything downstream of the mask is bf16-safe: bf16 q/k/v/scores/
    attn-weights/out-proj measured 4.1e-3 total vs jax ref (5x margin).
    bf16 (vs baseline's float32r) buys FWL fast weight loads on PE,
    2-4x DVE modes, and half the DMA bytes.

Performance structure (per core, from perfetto traces):
  - z stream: 256 steps (1 query row each) of [T-tile produce on DVE
    (~805ns) or ACT (~1054ns)] + 2 fp32 matmul calls (wsel delta trick,
    4-way col-tiled, 4 LDW + 4 MM instrs/step, LDW ~110ns each).
    Pace ~470-490ns/step, co-bound by dual-engine T production (~456)
    and the LDW chain (~440). This is near the structural floor: fp32
    MMs cannot share LDWs in bass, and exact T production is elementwise
    fp32 work on DVE/ACT (tensor_scalar runs 2 elem/cyc/lane via 2x_2P).
  - All other work (QKV projections, scores, softmax, AV, out-proj) is
    emitted as "fillers" interleaved into the two z-block windows so PE/
    DVE/ACT gaps between phases vanish.
  - Input DMAs are spread across 5 engine queues (sync/tensor/gpsimd/
    scalar/vector) instead of serializing ~23us on sync.
"""

import sys

if "/opt/trn_rl_repo" not in sys.path:
    sys.path.insert(0, "/opt/trn_rl_repo")

import numpy as np
import ml_dtypes

import concourse.bass as bass  # noqa: F401
import concourse.tile as tile
from concourse import bacc, mybir
from concourse.bass_utils import run_bass_kernel_spmd

F32 = mybir.dt.float32
BF16 = mybir.dt.bfloat16
AL = mybir.AluOpType
AF = mybir.ActivationFunctionType

B, S, D = 2, 1024, 512
DH = D // 2          # 256 per-head dim
HID = 128            # predictor hidden
NCHUNK = S // 4      # 256 query rows per core
P = 128

# knobs (test.py may override before first kernel() call)
CONFIG = {
    "trace": False,
    "tmpdir": None,
    # T-producer rotation per 16 rows: 'v'=DVE, 's'=ACT
    "tpat": ["v", "s", "v", "s", "v", "v", "s", "v", "s", "v", "v", "s", "v", "s", "v", "s"],
    "t_bufs": 24,
    "fill_start": 4,   # first z step that may carry a filler
    "fill_end": 124,   # last z step that may carry a filler
}

_STATE = {}


def _emit(tc, nc, t):
    sl512 = [slice(0, 512), slice(512, 1024)]

    with tc.tile_pool(name="big", bufs=1) as big:
        # ---- persistent residents ----
        # packed constants: each input tensor lands with ONE dma (the DMA
        # engines cost ~600ns/instruction regardless of size)
        mqk_s = big.tile([P, 4, 2 * HID], F32, name="mqk_s")   # [mq | mk]
        cf_s = big.tile([P, 66], F32, name="cf_s")             # b1|thr|wsel32
        b1_s = cf_s[:, 0:1]
        thr_s = cf_s[:, 1:2]
        wsel_s = cf_s[:, 2:66]
        cb_s = big.tile([1, D + P], BF16, name="cb_s")          # bo | ones
        bo_s = cb_s[:, 0:D]
        one_s = cb_s[:, D : D + P]
        ident = big.tile([P, P], BF16, name="ident")

        at_s = big.tile([P, NCHUNK], F32, name="at_s")    # a^T + b1, [h, i]
        nat_s = big.tile([P, NCHUNK], F32, name="nat_s")  # -(a^T + b1)
        ct_s = big.tile([P, S], F32, name="ct_s")          # c^T, [h, j]

        xbq_s = big.tile([P, 4, NCHUNK], BF16, name="xbq_s")  # x^T qchunk bf16
        xbt_s = big.tile([P, 4, S], BF16, name="xbt_s")       # x^T bf16
        wqb_s = big.tile([P, 4, D], BF16, name="wqb_s")       # Wq/16 bf16
        wkb_s = big.tile([P, 4, D], BF16, name="wkb_s")
        wvb_s = big.tile([P, 4, D], BF16, name="wvb_s")
        wob_s = big.tile([P, 4, D], BF16, name="wob_s")
        qt_s = big.tile([P, 4, NCHUNK], BF16, name="qt_s")  # q^T/16 [dout, i]
        kt_s = big.tile([P, 4, S], BF16, name="kt_s")       # k^T [dout, j]
        v_s = big.tile([P, 8, D], BF16, name="v_s")         # v [j(8 tiles), d]
        otr_s = big.tile([P, 4, NCHUNK], BF16, name="otr_s")  # out^T [d, i]
        mask_s = [big.tile([P, S], BF16, name=f"mask{b_}") for b_ in range(2)]

        # ---- input DMAs: one per tensor, spread over the 3 DMA queues ----
        nc.gpsimd.dma_start(cf_s[:], t["constsf"])
        nc.gpsimd.dma_start(mqk_s[:], t["mqk"])
        nc.scalar.dma_start(cb_s[:], t["constsb"])
        nc.scalar.dma_start(ident[:], t["identb"])

        with (
            tc.tile_pool(name="pjp", bufs=1, space="PSUM") as pjp,
            tc.tile_pool(name="zps", bufs=1, space="PSUM") as zpsp,
        ):
            # ---------------- stage A (exact a/c; transient f32 staging) ----
            with tc.tile_pool(name="stageA", bufs=1) as sa:
                xq_s = sa.tile([P, 4, NCHUNK], F32, name="xq_s")
                xt_s = sa.tile([P, 4, S], F32, name="xt_s")
                # x fp32 first (a/c critical path), then bf16 x, then weights
                nc.sync.dma_start(xq_s[:], t["xqT"])
                nc.sync.dma_start(xt_s[:], t["xT"])
                nc.gpsimd.dma_start(xbq_s[:], t["xbqT"])
                nc.gpsimd.dma_start(xbt_s[:], t["xbT"])
                nc.scalar.dma_start(wqb_s[:], t["wq"])
                nc.gpsimd.dma_start(wkb_s[:], t["wk"])
                nc.scalar.dma_start(wvb_s[:], t["wv"])
                nc.gpsimd.dma_start(wob_s[:], t["wo"])

                # exact a (fp32 matmuls, 2-way col-tiled so the 2-pass fp32
                # streams pipeline across array halves)
                at_ps = pjp.tile([P, NCHUNK], F32, tag="vps", bufs=2, name="at_ps")
                for dt_ in range(4):
                    for g2 in range(2):
                        nc.tensor.matmul(
                            at_ps[64 * g2 : 64 * (g2 + 1), :],
                            mqk_s[:, dt_, 64 * g2 : 64 * (g2 + 1)],
                            xq_s[:, dt_, :],
                            start=(dt_ == 0), stop=(dt_ == 3),
                            tile_position=(0, 64 * g2),
                            skip_group_check=True,
                        )
                nc.vector.tensor_scalar(at_s[:], at_ps[:], b1_s[:], None, AL.add)
                nc.vector.tensor_scalar(nat_s[:], at_s[:], -1.0, None, AL.mult)

                # exact c (fp32 matmuls, 2-way col-tiled)
                for jc in range(2):
                    ct_ps = pjp.tile([P, 512], F32, tag="vps", bufs=2, name="ct_ps")
                    for dt_ in range(4):
                        for g2 in range(2):
                            nc.tensor.matmul(
                                ct_ps[64 * g2 : 64 * (g2 + 1), :],
                                mqk_s[:, dt_, 128 + 64 * g2 : 128 + 64 * (g2 + 1)],
                                xt_s[:, dt_, sl512[jc]],
                                start=(dt_ == 0), stop=(dt_ == 3),
                                tile_position=(0, 64 * g2),
                                skip_group_check=True,
                            )
                    nc.scalar.copy(ct_s[:, sl512[jc]], ct_ps[:])

            # ---------------- z blocks with interleaved fillers ------------
            with (
                tc.tile_pool(name="Tp", bufs=CONFIG["t_bufs"]) as Tp,
                tc.tile_pool(name="work", bufs=2) as work,
            ):
                # ---- filler pieces (bf16 PE work + copies) ----
                def f_qt(dout):
                    def go():
                        qt_ps = pjp.tile([P, NCHUNK], F32, tag="vps", bufs=2, name="qt_ps")
                        for dt_ in range(4):
                            nc.tensor.matmul(
                                qt_ps[:], wqb_s[:, dt_, 128 * dout : 128 * (dout + 1)],
                                xbq_s[:, dt_, :], start=(dt_ == 0), stop=(dt_ == 3),
                            )
                        if dout % 2 == 0:
                            nc.vector.tensor_copy(qt_s[:, dout, :], qt_ps[:])
                        else:
                            nc.scalar.copy(qt_s[:, dout, :], qt_ps[:])
                    return go

                def f_kt(dout, jc):
                    def go():
                        kt_ps = pjp.tile([P, 512], F32, tag="vps", bufs=2, name="kt_ps")
                        for dt_ in range(4):
                            nc.tensor.matmul(
                                kt_ps[:],
                                wkb_s[:, dt_, 128 * dout : 128 * (dout + 1)],
                                xbt_s[:, dt_, sl512[jc]],
                                start=(dt_ == 0), stop=(dt_ == 3),
                            )
                        if (dout + jc) % 2 == 0:
                            nc.vector.tensor_copy(kt_s[:, dout, sl512[jc]], kt_ps[:])
                        else:
                            nc.scalar.copy(kt_s[:, dout, sl512[jc]], kt_ps[:])
                    return go

                def f_v(jt):
                    def go():
                        v_ps = pjp.tile([P, D], F32, tag="vps", bufs=2, name="v_ps")
                        for dt_ in range(4):
                            nc.tensor.matmul(
                                v_ps[:], xbt_s[:, dt_, 128 * jt : 128 * (jt + 1)],
                                wvb_s[:, dt_, :], start=(dt_ == 0), stop=(dt_ == 3),
                            )
                        if jt % 2 == 0:
                            nc.vector.tensor_copy(v_s[:, jt, :], v_ps[:])
                        else:
                            nc.scalar.copy(v_s[:, jt, :], v_ps[:])
                    return go

                # per-head attnT accumulators (written in ti halves)
                att_sb = [
                    [
                        work.tile([P, NCHUNK], BF16, tag="attnT", bufs=16,
                                  name=f"a_sb{h}_{jt}")
                        for jt in range(8)
                    ]
                    for h in range(2)
                ]

                def attn_half_pieces(h, ti):
                    """softmax(masked scores) for rows [128*ti,128*(ti+1)) of
                    head h, transposed into att_sb[h][*][:, 128*ti:]. Returns
                    a list of closures (emission units)."""
                    st = {}

                    def sc(jc):
                        def go():
                            scp = pjp.tile([P, 512], F32, tag="vps", bufs=2,
                                           name=f"sc{h}{ti}{jc}")
                            st[f"sc{jc}"] = scp
                            for dt_ in range(2):
                                nc.tensor.matmul(
                                    scp[:],
                                    qt_s[:, 2 * h + dt_, 128 * ti : 128 * (ti + 1)],
                                    kt_s[:, 2 * h + dt_, sl512[jc]],
                                    start=(dt_ == 0), stop=(dt_ == 1),
                                )
                        return go

                    def ex(jc):
                        def go():
                            if jc == 0:
                                st["e"] = work.tile([P, S], BF16, tag="e", name="e")
                            # scores/16 bounded (|sc|<~7): exp cannot overflow,
                            # rowmax subtraction dropped (identical result)
                            nc.scalar.activation(
                                st["e"][:, sl512[jc]], st[f"sc{jc}"][:], AF.Exp
                            )
                        return go

                    def sm():
                        e = st["e"]
                        em = work.tile([P, S], BF16, tag="em", name="em")
                        st["em"] = em
                        ssum = work.tile([P, 1], F32, tag="ssum", name="ssum")
                        nc.vector.scalar_tensor_tensor(
                            em[:], e[:], 0.0, mask_s[ti][:], AL.add, AL.mult,
                            accum_out=ssum[:],
                        )
                        # fully-masked rows: reference = uniform 1/1024.
                        # ind=[s==0]; attn = (em + ind) / (s + 1024*ind)
                        ind = work.tile([P, 1], F32, tag="ind", name="ind")
                        nc.vector.tensor_scalar(ind[:], ssum[:], 0.0, None, AL.is_equal)
                        s2 = work.tile([P, 1], F32, tag="s2", name="s2")
                        nc.vector.tensor_scalar(s2[:], ind[:], 1024.0, ssum[:], AL.mult, AL.add)
                        rinv = work.tile([P, 1], F32, tag="rinv", name="rinv")
                        nc.vector.reciprocal(rinv[:], s2[:])
                        nc.vector.tensor_scalar(em[:], em[:], ind[:], rinv[:], AL.add, AL.mult)

                    def tp(half):
                        def go():
                            for jt in range(4 * half, 4 * half + 4):
                                tp_ps = pjp.tile([P, P], BF16, tag="tp", bufs=2, name="tp_ps")
                                nc.tensor.transpose(
                                    tp_ps[:], st["em"][:, 128 * jt : 128 * (jt + 1)], ident[:]
                                )
                                dst = att_sb[h][jt][:, 128 * ti : 128 * (ti + 1)]
                                if jt % 2 == 0:
                                    nc.vector.tensor_copy(dst, tp_ps[:])
                                else:
                                    nc.scalar.copy(dst, tp_ps[:])
                        return go

                    return [sc(0), ex(0), sc(1), ex(1), sm, tp(0), tp(1)]

                def emit_av(h):
                    for dt_ in range(2):
                        ot_ps = pjp.tile([P, NCHUNK], F32, tag="vps", bufs=2, name="ot_ps")
                        for jt in range(8):
                            nc.tensor.matmul(
                                ot_ps[:],
                                v_s[:, jt, 256 * h + 128 * dt_ : 256 * h + 128 * (dt_ + 1)],
                                att_sb[h][jt][:],
                                start=(jt == 0), stop=(jt == 7),
                            )
                        if dt_ == 0:
                            nc.vector.tensor_copy(otr_s[:, 2 * h + dt_, :], ot_ps[:])
                        else:
                            nc.scalar.copy(otr_s[:, 2 * h + dt_, :], ot_ps[:])

                def emit_zblock(blk, fillers=()):
                    # z accumulation for 128 query rows; col groups MUST cycle
                    # (g=step%4): serial same-group fp32 mms run at 2cyc/col,
                    # cycling pipelines passes across the 4 array quadrants.
                    zp = zpsp.tile([P, S], F32, tag="z", bufs=2, name=f"zp{blk}")
                    nf = len(fillers)
                    fi = 0
                    a_, b_ = CONFIG["fill_start"], CONFIG["fill_end"]
                    for step in range(128):
                        k, g = step // 4, step % 4
                        i = 32 * g + k
                        ii = blk * 128 + i
                        T = Tp.tile([P, S], F32, tag="T", name=f"T{ii}")
                        eng = CONFIG["tpat"][ii % 16]
                        if eng == "v":
                            # relu(ct + a) as max(ct, -a) + a  ((add,max) hits a
                            # slow DVE path; (max,add) measured ~805ns)
                            nc.vector.tensor_scalar(
                                T[:], ct_s[:], nat_s[:, ii : ii + 1],
                                at_s[:, ii : ii + 1], AL.max, AL.add,
                            )
                        else:
                            nc.scalar.activation(
                                T[:], ct_s[:], AF.Relu, bias=at_s[:, ii : ii + 1]
                            )
                        for jc in range(2):
                            nc.tensor.matmul(
                                zp[32 * g : 32 * g + 32, sl512[jc]],
                                wsel_s[:, 32 - k : 64 - k],
                                T[:, sl512[jc]],
                                start=(k == 0), stop=(k == 31),
                                tile_position=(0, 32 * g),
                                skip_group_check=True,
                            )
                        while fi < nf and a_ + ((b_ - a_) * fi) // max(nf - 1, 1) <= step:
                            fillers[fi]()
                            fi += 1
                    while fi < nf:
                        fillers[fi]()
                        fi += 1
                    nc.vector.tensor_scalar(
                        mask_s[blk][:], zp[:], thr_s[:], None, AL.is_gt,
                    )

                # ---- emission schedule ----
                blk0_fill = (
                    [f_qt(d) for d in range(4)]
                    + [f_kt(d, jc) for d in range(4) for jc in range(2)]
                    + [f_v(jt) for jt in range(8)]
                )
                emit_zblock(0, blk0_fill)

                blk1_fill = attn_half_pieces(0, 0) + attn_half_pieces(1, 0)
                emit_zblock(1, blk1_fill)

                # ---- tail: attn ti=1 for both heads, AV, out-proj ----
                p0 = attn_half_pieces(0, 1)
                p1 = attn_half_pieces(1, 1)
                p0[0](); p0[1](); p0[2](); p0[3]()   # sc0/exp0/sc1/exp1 head0
                p1[0](); p1[1]()                     # head1 sc0/exp0
                p0[4]()                              # head0 softmax (DVE)
                p1[2](); p1[3]()                     # head1 sc1/exp1
                p0[5](); p0[6]()                     # head0 transposes
                p1[4]()                              # head1 softmax
                emit_av(0)
                p1[5](); p1[6]()                     # head1 transposes
                emit_av(1)

                # stage F: output projection (bf16; bo/ones exact in bf16)
                for ti in range(2):
                    o_ps = pjp.tile([P, D], F32, tag="vps", bufs=2, name="o_ps")
                    nc.tensor.matmul(o_ps[:], one_s[:], bo_s[:], start=True, stop=False)
                    for dt_ in range(4):
                        nc.tensor.matmul(
                            o_ps[:], otr_s[:, dt_, 128 * ti : 128 * (ti + 1)],
                            wob_s[:, dt_, :], start=False, stop=(dt_ == 3),
                        )
                    o_sb = work.tile([P, D], F32, tag="osb", bufs=2, name="o_sb")
                    nc.vector.tensor_copy(o_sb[:], o_ps[:])
                    nc.sync.dma_start(t["out"][128 * ti : 128 * (ti + 1), :], o_sb[:])


def _build():
    if "nc" in _STATE:
        return _STATE["nc"]
    nc = bacc.Bacc(
        "TRN2", target_bir_lowering=False, debug=False, enable_asserts=True,
        num_devices=8,
    )
    t = {}
    t["xT"] = nc.dram_tensor("xT", [P, 4, S], F32, kind="ExternalInput").ap()
    t["xqT"] = nc.dram_tensor("xqT", [P, 4, NCHUNK], F32, kind="ExternalInput").ap()
    t["xbT"] = nc.dram_tensor("xbT", [P, 4, S], BF16, kind="ExternalInput").ap()
    t["xbqT"] = nc.dram_tensor("xbqT", [P, 4, NCHUNK], BF16, kind="ExternalInput").ap()
    t["wq"] = nc.dram_tensor("wq", [P, 4, D], BF16, kind="ExternalInput").ap()
    t["wk"] = nc.dram_tensor("wk", [P, 4, D], BF16, kind="ExternalInput").ap()
    t["wv"] = nc.dram_tensor("wv", [P, 4, D], BF16, kind="ExternalInput").ap()
    t["wo"] = nc.dram_tensor("wo", [P, 4, D], BF16, kind="ExternalInput").ap()
    t["mqk"] = nc.dram_tensor("mqk", [P, 4, 2 * HID], F32, kind="ExternalInput").ap()
    t["constsf"] = nc.dram_tensor("constsf", [P, 66], F32, kind="ExternalInput").ap()
    t["constsb"] = nc.dram_tensor("constsb", [1, D + P], BF16, kind="ExternalInput").ap()
    t["identb"] = nc.dram_tensor("identb", [P, P], BF16, kind="ExternalInput").ap()
    t["out"] = nc.dram_tensor("out", [NCHUNK, D], F32, kind="ExternalOutput").ap()

    with tile.TileContext(nc) as tc:
        _emit(tc, nc, t)
    nc.compile()
    _STATE["nc"] = nc
    return nc


def _prep_in_maps(inputs):
    bf16 = ml_dtypes.bfloat16
    x = np.ascontiguousarray(np.asarray(inputs["x"], np.float32))
    Wq = np.asarray(inputs["Wq"], np.float32)
    Wk = np.asarray(inputs["Wk"], np.float32)
    Wv = np.asarray(inputs["Wv"], np.float32)
    Wo = np.asarray(inputs["Wo"], np.float32)
    bo = np.asarray(inputs["bo"], np.float32)
    W1 = np.asarray(inputs["W1"], np.float64)
    b1 = np.asarray(inputs["b1"], np.float32)
    W2 = np.asarray(inputs["W2"], np.float32)
    b2 = np.asarray(inputs["b2"], np.float32)

    wq_m = 0.5 * (Wq[:, :DH].astype(np.float64) + Wq[:, DH:].astype(np.float64))
    wk_m = 0.5 * (Wk[:, :DH].astype(np.float64) + Wk[:, DH:].astype(np.float64))
    Mq = np.ascontiguousarray((wq_m @ W1[:DH]).astype(np.float32))
    Mk = np.ascontiguousarray((wk_m @ W1[DH:]).astype(np.float32))

    def chunk(a):
        # [D, N] -> [P, 4, N]: partition-chunked layout for one-shot DMA
        return np.ascontiguousarray(a.reshape(4, P, -1).transpose(1, 0, 2))

    constsf = np.zeros((P, 66), np.float32)
    constsf[:, 0] = b1
    constsf[:, 1] = -float(b2[0])
    constsf[:, 2 + 32] = W2[:, 0]          # wsel32 window buffer
    constsb = np.zeros((1, D + P), bf16)
    constsb[0, :D] = bo.astype(bf16)
    constsb[0, D:] = np.ones(P, bf16)

    shared = dict(
        wq=chunk((Wq / 16.0).astype(bf16)),
        wk=chunk(Wk.astype(bf16)),
        wv=chunk(Wv.astype(bf16)),
        wo=chunk(Wo.astype(bf16)),
        mqk=chunk(np.concatenate([Mq, Mk], axis=1)),
        constsf=constsf, constsb=constsb,
        identb=np.eye(P, dtype=bf16),
    )
    in_maps = []
    xT = [np.ascontiguousarray(x[b].T) for b in range(B)]
    for c in range(8):
        b, i0 = c // 4, (c % 4) * NCHUNK
        m = dict(shared)
        m["xT"] = chunk(xT[b])
        m["xbT"] = chunk(xT[b].astype(bf16))
        xq = x[b, i0 : i0 + NCHUNK].T
        m["xqT"] = chunk(xq)
        m["xbqT"] = chunk(xq.astype(bf16))
        in_maps.append(m)
    return in_maps


def kernel(**inputs):
    nc = _build()
    in_maps = _prep_in_maps(inputs)
    res = run_bass_kernel_spmd(
        nc, in_maps, core_ids=list(range(8)),
        trace=CONFIG["trace"], tmpdir=CONFIG["tmpdir"],
    )
    _STATE["last_result"] = res
    out = np.empty((B, S, D), np.float32)
    for c in range(8):
        b, i0 = c // 4, (c % 4) * NCHUNK
        out[b, i0 : i0 + NCHUNK] = res.results[c]["out"]
    return out


# revision 13
# speedup vs baseline: 1.1208x; 1.0549x over previous
"""AdaptiveSparsityAttention TRN2 kernel (8 NeuronCores, SPMD data-parallel).

Problem (B=2, S=1024, D=512, H=2 heads, dh=256, hidden=128):
  q,k,v = x@Wq, x@Wk, x@Wv (split 2 heads); scores = q@k^T/16
  a_i = q_mean@W1[:dh]+b1, c_j = k_mean@W1[dh:]
  z[i,j] = W2 . relu(a_i + c_j)          (sigmoid(z+b2)>0.5  <=>  z > -b2)
  attn = softmax(mask(scores));  out = (attn@v)@Wo + bo

Sharding: 8 cores = 2 batches x 4 query-chunks of 256 rows. Each core
computes its output chunk fully locally, no collectives.

Precision structure (measured, numpy sims):
  - z MUST be fp32-exact: z std 0.022, ~46% fill, threshold 0. bf16-level
    z error flips ~0.1% of mask bits -> 5.8e-2 L2 (FAILS 2e-2 gate);
    fp16 -> 1.5e-2 (too marginal). fp32 mask matches jax ref at 1.4e-3.
  - Everything downstream of the mask is bf16-safe: bf16 q/k/v/scores/
    attn-weights/out-proj measured 4.1e-3 total vs jax ref (5x margin).
    bf16 (vs baseline's float32r) buys FWL fast weight loads on PE,
    2-4x DVE modes, and half the DMA bytes.

Performance structure (per core, from perfetto traces):
  - z stream: 256 steps (1 query row each) of [T-tile produce on DVE
    (~805ns) or ACT (~1054ns)] + 2 fp32 matmul calls (wsel delta trick,
    4-way col-tiled, 4 LDW + 4 MM instrs/step, LDW ~110ns each).
    Pace ~470-490ns/step, co-bound by dual-engine T production (~456)
    and the LDW chain (~440). This is near the structural floor: fp32
    MMs cannot share LDWs in bass, and exact T production is elementwise
    fp32 work on DVE/ACT (tensor_scalar runs 2 elem/cyc/lane via 2x_2P).
  - All other work (QKV projections, scores, softmax, AV, out-proj) is
    emitted as "fillers" interleaved into the two z-block windows so PE/
    DVE/ACT gaps between phases vanish.
  - Input DMAs are spread across 5 engine queues (sync/tensor/gpsimd/
    scalar/vector) instead of serializing ~23us on sync.
"""

import sys

if "/opt/trn_rl_repo" not in sys.path:
    sys.path.insert(0, "/opt/trn_rl_repo")

import numpy as np
import ml_dtypes

import concourse.bass as bass  # noqa: F401
import concourse.tile as tile
from concourse import bacc, mybir
from concourse.bass_utils import run_bass_kernel_spmd

F32 = mybir.dt.float32
BF16 = mybir.dt.bfloat16
AL = mybir.AluOpType
AF = mybir.ActivationFunctionType

B, S, D = 2, 1024, 512
DH = D // 2          # 256 per-head dim
HID = 128            # predictor hidden
NCHUNK = S // 4      # 256 query rows per core
P = 128

# knobs (test.py may override before first kernel() call)
CONFIG = {
    "trace": False,
    "tmpdir": None,
    # T-producer rotation per 16 rows: 'v'=DVE, 's'=ACT
    "tpat": ["v", "s", "v", "s", "v", "v", "s", "v", "s", "v", "v", "s", "v", "s", "v", "s"],
    "t_bufs": 24,
    "fill_start": 4,   # first z step that may carry a filler
    "fill_end": 124,   # last z step that may carry a filler
}

_STATE = {}


def _emit(tc, nc, t):
    sl512 = [slice(0, 512), slice(512, 1024)]

    with tc.tile_pool(name="big", bufs=1) as big:
        # ---- persistent residents ----
        mqk_s = big.tile([P, 4, 2 * HID], F32, name="mqk_s")   # [mq | mk]
        cf_s = big.tile([P, 66], F32, name="cf_s")             # b1|thr|wsel32
        b1_s = cf_s[:, 0:1]
        thr_s = cf_s[:, 1:2]
        wsel_s = cf_s[:, 2:66]
        cb_s = big.tile([1, D + P], BF16, name="cb_s")          # bo | ones
        bo_s = cb_s[:, 0:D]
        one_s = cb_s[:, D : D + P]
        ident = big.tile([P, P], BF16, name="ident")

        at_s = big.tile([P, NCHUNK], F32, name="at_s")    # a^T + b1, [h, i]
        nat_s = big.tile([P, NCHUNK], F32, name="nat_s")  # -(a^T + b1)
        ct_s = big.tile([P, S], F32, name="ct_s")          # c^T, [h, j]

        xbt_s = big.tile([P, 4, S], BF16, name="xbt_s")       # x^T bf16
        wqb_s = big.tile([P, 4, D], BF16, name="wqb_s")       # Wq/16 bf16
        wkb_s = big.tile([P, 4, D], BF16, name="wkb_s")
        wvb_s = big.tile([P, 4, D], BF16, name="wvb_s")
        wob_s = big.tile([P, 4, D], BF16, name="wob_s")
        qt_s = big.tile([P, 4, NCHUNK], BF16, name="qt_s")  # q^T/16 [dout, i]
        kt_s = big.tile([P, 4, S], BF16, name="kt_s")       # k^T [dout, j]
        v_s = big.tile([P, 8, D], BF16, name="v_s")         # v [j(8 tiles), d]
        otr_s = big.tile([P, 4, NCHUNK], BF16, name="otr_s")  # out^T [d, i]
        mask_s = [big.tile([P, S], BF16, name=f"mask{b_}") for b_ in range(2)]

        with (
            tc.tile_pool(name="pjp", bufs=1, space="PSUM") as pjp,
            tc.tile_pool(name="zps", bufs=1, space="PSUM") as zpsp,
        ):
            # -------- stage A: DMAs + exact a/c (transient f32 staging) ----
            with tc.tile_pool(name="stageA", bufs=1) as sa:
                xt_s = sa.tile([P, 4, S], F32, name="xt_s")
                # x fp32 in j-quarters across 2 queues: quarter 0 holds the
                # (host-reordered) query chunk, so `a` starts ~3.5us in.
                # gpsimd gets mqk (other a/c dependency) first.
                nc.gpsimd.dma_start(cf_s[:], t["constsf"])
                nc.gpsimd.dma_start(mqk_s[:], t["mqk"])
                for qtr, q in [(0, nc.sync), (1, nc.scalar), (2, nc.sync), (3, nc.scalar)]:
                    sl = slice(256 * qtr, 256 * (qtr + 1))
                    q.dma_start(xt_s[:, :, sl], t["xT"][:, :, sl])
                nc.gpsimd.dma_start(xbt_s[:], t["xbT"])
                nc.gpsimd.dma_start(wkb_s[:], t["wk"])
                nc.sync.dma_start(wqb_s[:], t["wq"])
                nc.scalar.dma_start(wvb_s[:], t["wv"])
                nc.scalar.dma_start(cb_s[:], t["constsb"])
                nc.scalar.dma_start(ident[:], t["identb"])
                nc.sync.dma_start(wob_s[:], t["wo"])

                # exact a (fp32 matmuls, 2-way col-tiled; query chunk = x
                # columns 0..255 thanks to the host-side reorder)
                at_ps = pjp.tile([P, NCHUNK], F32, tag="vps", bufs=2, name="at_ps")
                for dt_ in range(4):
                    for g2 in range(2):
                        nc.tensor.matmul(
                            at_ps[64 * g2 : 64 * (g2 + 1), :],
                            mqk_s[:, dt_, 64 * g2 : 64 * (g2 + 1)],
                            xt_s[:, dt_, 0:NCHUNK],
                            start=(dt_ == 0), stop=(dt_ == 3),
                            tile_position=(0, 64 * g2),
                            skip_group_check=True,
                        )
                nc.vector.tensor_scalar(at_s[:], at_ps[:], b1_s[:], None, AL.add)
                nc.vector.tensor_scalar(nat_s[:], at_s[:], -1.0, None, AL.mult)

                # exact c (fp32, 2-way col-tiled, per j-quarter as DMAs land)
                for qtr in range(4):
                    sl = slice(256 * qtr, 256 * (qtr + 1))
                    ct_ps = pjp.tile([P, 256], F32, tag="vps", bufs=2, name="ct_ps")
                    for dt_ in range(4):
                        for g2 in range(2):
                            nc.tensor.matmul(
                                ct_ps[64 * g2 : 64 * (g2 + 1), :],
                                mqk_s[:, dt_, 128 + 64 * g2 : 128 + 64 * (g2 + 1)],
                                xt_s[:, dt_, sl],
                                start=(dt_ == 0), stop=(dt_ == 3),
                                tile_position=(0, 64 * g2),
                                skip_group_check=True,
                            )
                    if qtr % 2 == 0:
                        nc.scalar.copy(ct_s[:, sl], ct_ps[:])
                    else:
                        nc.vector.tensor_copy(ct_s[:, sl], ct_ps[:])

            # ---------------- z blocks / mid / tail ------------------------
            with (
                tc.tile_pool(name="Tp", bufs=CONFIG["t_bufs"]) as Tp,
                tc.tile_pool(name="work", bufs=2) as work,
            ):
                att_sb = [
                    [
                        work.tile([P, NCHUNK], BF16, tag="attnT", bufs=16,
                                  name=f"a_sb{h}_{jt}")
                        for jt in range(8)
                    ]
                    for h in range(2)
                ]
                # exp(scores) for all 4 (head, ti) pairs: mask-independent,
                # computed in the mid gap; masked in softmax_finish later.
                e_sb = [
                    [work.tile([P, S], BF16, tag="e", bufs=4, name=f"e{h}_{ti}")
                     for ti in range(2)]
                    for h in range(2)
                ]

                def emit_zblock(blk):
                    # z accumulation for 128 query rows; col groups MUST cycle
                    # (g=step%4): serial same-group fp32 mms run at 2cyc/col,
                    # cycling pipelines passes across the 4 array quadrants.
                    # Keep this stream PURE: a full-array matmul inserted here
                    # drains the quadrant pipeline (~1us each).
                    zp = zpsp.tile([P, S], F32, tag="z", bufs=2, name=f"zp{blk}")
                    for step in range(128):
                        k, g = step // 4, step % 4
                        i = 32 * g + k
                        ii = blk * 128 + i
                        T = Tp.tile([P, S], F32, tag="T", name=f"T{ii}")
                        eng = CONFIG["tpat"][ii % 16]
                        if eng == "v":
                            # relu(ct + a) as max(ct, -a) + a  ((add,max) hits
                            # a slow DVE path; (max,add) measured ~805ns)
                            nc.vector.tensor_scalar(
                                T[:], ct_s[:], nat_s[:, ii : ii + 1],
                                at_s[:, ii : ii + 1], AL.max, AL.add,
                            )
                        else:
                            nc.scalar.activation(
                                T[:], ct_s[:], AF.Relu, bias=at_s[:, ii : ii + 1]
                            )
                        for jc in range(2):
                            nc.tensor.matmul(
                                zp[32 * g : 32 * g + 32, sl512[jc]],
                                wsel_s[:, 32 - k : 64 - k],
                                T[:, sl512[jc]],
                                start=(k == 0), stop=(k == 31),
                                tile_position=(0, 32 * g),
                                skip_group_check=True,
                            )
                    nc.vector.tensor_scalar(
                        mask_s[blk][:], zp[:], thr_s[:], None, AL.is_gt,
                    )

                def emit_qkv():
                    # projections: dense bf16 PE chains (FWL weight loads)
                    for dout in range(4):
                        qt_ps = pjp.tile([P, NCHUNK], F32, tag="vps", bufs=2, name="qt_ps")
                        for dt_ in range(4):
                            nc.tensor.matmul(
                                qt_ps[:], wqb_s[:, dt_, 128 * dout : 128 * (dout + 1)],
                                xbt_s[:, dt_, 0:NCHUNK], start=(dt_ == 0), stop=(dt_ == 3),
                            )
                        if dout % 2 == 0:
                            nc.vector.tensor_copy(qt_s[:, dout, :], qt_ps[:])
                        else:
                            nc.scalar.copy(qt_s[:, dout, :], qt_ps[:])
                    for dout in range(4):
                        for jc in range(2):
                            kt_ps = pjp.tile([P, 512], F32, tag="vps", bufs=2, name="kt_ps")
                            for dt_ in range(4):
                                nc.tensor.matmul(
                                    kt_ps[:],
                                    wkb_s[:, dt_, 128 * dout : 128 * (dout + 1)],
                                    xbt_s[:, dt_, sl512[jc]],
                                    start=(dt_ == 0), stop=(dt_ == 3),
                                )
                            if (dout + jc) % 2 == 0:
                                nc.vector.tensor_copy(kt_s[:, dout, sl512[jc]], kt_ps[:])
                            else:
                                nc.scalar.copy(kt_s[:, dout, sl512[jc]], kt_ps[:])
                    for jt in range(8):
                        v_ps = pjp.tile([P, D], F32, tag="vps", bufs=2, name="v_ps")
                        for dt_ in range(4):
                            nc.tensor.matmul(
                                v_ps[:], xbt_s[:, dt_, 128 * jt : 128 * (jt + 1)],
                                wvb_s[:, dt_, :], start=(dt_ == 0), stop=(dt_ == 3),
                            )
                        if jt % 2 == 0:
                            nc.vector.tensor_copy(v_s[:, jt, :], v_ps[:])
                        else:
                            nc.scalar.copy(v_s[:, jt, :], v_ps[:])

                def emit_scores(h, ti):
                    # scores + exp for rows [128*ti,128*(ti+1)) of head h.
                    # Mask-independent: runs in the mid gap for both ti.
                    for jc in range(2):
                        scp = pjp.tile([P, 512], F32, tag="vps", bufs=2,
                                       name=f"sc{h}{ti}{jc}")
                        for dt_ in range(2):
                            nc.tensor.matmul(
                                scp[:],
                                qt_s[:, 2 * h + dt_, 128 * ti : 128 * (ti + 1)],
                                kt_s[:, 2 * h + dt_, sl512[jc]],
                                start=(dt_ == 0), stop=(dt_ == 1),
                            )
                        # scores/16 bounded (|sc|<~7): exp cannot overflow,
                        # rowmax subtraction dropped (identical result)
                        nc.scalar.activation(
                            e_sb[h][ti][:, sl512[jc]], scp[:], AF.Exp
                        )

                def softmax_finish(h, ti):
                    # em = (e*mask + ind)/(sum + 1024*ind); needs mask_s[ti]
                    e = e_sb[h][ti]
                    em = work.tile([P, S], BF16, tag="em", name="em")
                    ssum = work.tile([P, 1], F32, tag="ssum", name="ssum")
                    nc.vector.scalar_tensor_tensor(
                        em[:], e[:], 0.0, mask_s[ti][:], AL.add, AL.mult,
                        accum_out=ssum[:],
                    )
                    ind = work.tile([P, 1], F32, tag="ind", name="ind")
                    nc.vector.tensor_scalar(ind[:], ssum[:], 0.0, None, AL.is_equal)
                    s2 = work.tile([P, 1], F32, tag="s2", name="s2")
                    nc.vector.tensor_scalar(s2[:], ind[:], 1024.0, ssum[:], AL.mult, AL.add)
                    rinv = work.tile([P, 1], F32, tag="rinv", name="rinv")
                    nc.vector.reciprocal(rinv[:], s2[:])
                    nc.vector.tensor_scalar(em[:], em[:], ind[:], rinv[:], AL.add, AL.mult)
                    return em

                def emit_transposes(h, ti, em):
                    for jt in range(8):
                        tp_ps = pjp.tile([P, P], BF16, tag="tp", bufs=2, name="tp_ps")
                        nc.tensor.transpose(
                            tp_ps[:], em[:, 128 * jt : 128 * (jt + 1)], ident[:]
                        )
                        dst = att_sb[h][jt][:, 128 * ti : 128 * (ti + 1)]
                        if jt % 2 == 0:
                            nc.vector.tensor_copy(dst, tp_ps[:])
                        else:
                            nc.scalar.copy(dst, tp_ps[:])

                def emit_attn(h, ti):
                    em = softmax_finish(h, ti)
                    emit_transposes(h, ti, em)

                def emit_av(h):
                    for dt_ in range(2):
                        ot_ps = pjp.tile([P, NCHUNK], F32, tag="vps", bufs=2, name="ot_ps")
                        for jt in range(8):
                            nc.tensor.matmul(
                                ot_ps[:],
                                v_s[:, jt, 256 * h + 128 * dt_ : 256 * h + 128 * (dt_ + 1)],
                                att_sb[h][jt][:],
                                start=(jt == 0), stop=(jt == 7),
                            )
                        if dt_ == 0:
                            nc.vector.tensor_copy(otr_s[:, 2 * h + dt_, :], ot_ps[:])
                        else:
                            nc.scalar.copy(otr_s[:, 2 * h + dt_, :], ot_ps[:])

                # ---- emission schedule ----
                emit_zblock(0)
                # mid gap: projections, all scores+exp, blk0-half softmax+
                # transposes. T producers run ahead into the deep pool.
                emit_qkv()
                emit_scores(0, 0)
                emit_scores(1, 0)
                emit_attn(0, 0)
                emit_scores(0, 1)
                emit_attn(1, 0)
                emit_scores(1, 1)

                emit_zblock(1)

                # tail: only mask1-dependent work remains
                emit_attn(0, 1)
                emit_av(0)
                emit_attn(1, 1)
                emit_av(1)

                # stage F: output projection (bf16; bo/ones exact in bf16)
                for ti in range(2):
                    o_ps = pjp.tile([P, D], F32, tag="vps", bufs=2, name="o_ps")
                    nc.tensor.matmul(o_ps[:], one_s[:], bo_s[:], start=True, stop=False)
                    for dt_ in range(4):
                        nc.tensor.matmul(
                            o_ps[:], otr_s[:, dt_, 128 * ti : 128 * (ti + 1)],
                            wob_s[:, dt_, :], start=False, stop=(dt_ == 3),
                        )
                    o_sb = work.tile([P, D], F32, tag="osb", bufs=2, name="o_sb")
                    nc.vector.tensor_copy(o_sb[:], o_ps[:])
                    nc.sync.dma_start(t["out"][128 * ti : 128 * (ti + 1), :], o_sb[:])


def _build():
    if "nc" in _STATE:
        return _STATE["nc"]
    nc = bacc.Bacc(
        "TRN2", target_bir_lowering=False, debug=False, enable_asserts=True,
        num_devices=8,
    )
    t = {}
    t["xT"] = nc.dram_tensor("xT", [P, 4, S], F32, kind="ExternalInput").ap()
    t["xbT"] = nc.dram_tensor("xbT", [P, 4, S], BF16, kind="ExternalInput").ap()
    t["wq"] = nc.dram_tensor("wq", [P, 4, D], BF16, kind="ExternalInput").ap()
    t["wk"] = nc.dram_tensor("wk", [P, 4, D], BF16, kind="ExternalInput").ap()
    t["wv"] = nc.dram_tensor("wv", [P, 4, D], BF16, kind="ExternalInput").ap()
    t["wo"] = nc.dram_tensor("wo", [P, 4, D], BF16, kind="ExternalInput").ap()
    t["mqk"] = nc.dram_tensor("mqk", [P, 4, 2 * HID], F32, kind="ExternalInput").ap()
    t["constsf"] = nc.dram_tensor("constsf", [P, 66], F32, kind="ExternalInput").ap()
    t["constsb"] = nc.dram_tensor("constsb", [1, D + P], BF16, kind="ExternalInput").ap()
    t["identb"] = nc.dram_tensor("identb", [P, P], BF16, kind="ExternalInput").ap()
    t["out"] = nc.dram_tensor("out", [NCHUNK, D], F32, kind="ExternalOutput").ap()

    with tile.TileContext(nc) as tc:
        _emit(tc, nc, t)
    nc.compile()
    _STATE["nc"] = nc
    return nc


def _prep_in_maps(inputs):
    bf16 = ml_dtypes.bfloat16
    x = np.ascontiguousarray(np.asarray(inputs["x"], np.float32))
    Wq = np.asarray(inputs["Wq"], np.float32)
    Wk = np.asarray(inputs["Wk"], np.float32)
    Wv = np.asarray(inputs["Wv"], np.float32)
    Wo = np.asarray(inputs["Wo"], np.float32)
    bo = np.asarray(inputs["bo"], np.float32)
    W1 = np.asarray(inputs["W1"], np.float64)
    b1 = np.asarray(inputs["b1"], np.float32)
    W2 = np.asarray(inputs["W2"], np.float32)
    b2 = np.asarray(inputs["b2"], np.float32)

    wq_m = 0.5 * (Wq[:, :DH].astype(np.float64) + Wq[:, DH:].astype(np.float64))
    wk_m = 0.5 * (Wk[:, :DH].astype(np.float64) + Wk[:, DH:].astype(np.float64))
    Mq = np.ascontiguousarray((wq_m @ W1[:DH]).astype(np.float32))
    Mk = np.ascontiguousarray((wk_m @ W1[DH:]).astype(np.float32))

    def chunk(a):
        # [D, N] -> [P, 4, N]: partition-chunked layout for one-shot DMA
        return np.ascontiguousarray(a.reshape(4, P, -1).transpose(1, 0, 2))

    constsf = np.zeros((P, 66), np.float32)
    constsf[:, 0] = b1
    constsf[:, 1] = -float(b2[0])
    constsf[:, 2 + 32] = W2[:, 0]          # wsel32 window buffer
    constsb = np.zeros((1, D + P), bf16)
    constsb[0, :D] = bo.astype(bf16)
    constsb[0, D:] = np.ones(P, bf16)

    shared = dict(
        wq=chunk((Wq / 16.0).astype(bf16)),
        wk=chunk(Wk.astype(bf16)),
        wv=chunk(Wv.astype(bf16)),
        wo=chunk(Wo.astype(bf16)),
        mqk=chunk(np.concatenate([Mq, Mk], axis=1)),
        constsf=constsf, constsb=constsb,
        identb=np.eye(P, dtype=bf16),
    )
    in_maps = []
    for c in range(8):
        b, i0 = c // 4, (c % 4) * NCHUNK
        m = dict(shared)
        # roll x columns so this core's query chunk sits at j=0..255; the
        # softmax result is invariant to a consistent j-permutation of
        # keys/values/mask, and it lets `a` start from the first x DMA.
        xr = np.roll(x[b].T, -i0, axis=1)
        m["xT"] = chunk(xr)
        m["xbT"] = chunk(xr.astype(bf16))
        in_maps.append(m)
    return in_maps


def kernel(**inputs):
    nc = _build()
    in_maps = _prep_in_maps(inputs)
    res = run_bass_kernel_spmd(
        nc, in_maps, core_ids=list(range(8)),
        trace=CONFIG["trace"], tmpdir=CONFIG["tmpdir"],
    )
    _STATE["last_result"] = res
    out = np.empty((B, S, D), np.float32)
    for c in range(8):
        b, i0 = c // 4, (c % 4) * NCHUNK
        out[b, i0 : i0 + NCHUNK] = res.results[c]["out"]
    return out
